# revision 26
# baseline (speedup 1.0000x reference)
"""BayesianIGCNet Trainium2 kernel.

Strategy (node-sharded dst-grid, device-resident statics, chained on-device
dispatches):
- Host (once, cached): sort edges by dst, deal nodes round-robin by degree
  rank into 32 buckets (8 cores x 4 streams), two-class pad each node's slot
  run (deg<=32 -> 32 slots, else 64) by duplicating a real edge (max is
  idempotent), precompute static per-slot L1 inputs and per-slot src grid
  indices; device_put all static arrays once (sharded over the 8 cores).
- Per call: upload only the sampled weights (~KB). Run 5 chained dispatches
  whose intermediates stay on device:
    bass(iter1) -> XLA gather (uc = c[slot_src]) -> bass(iter2) -> gather ->
    bass(iter3) -> fetch [32,3200] c values.
- Bass NEFF (one IGConv layer): L1/L2 matmuls with persistent block-diagonal
  stationaries, ACT relu, DVE segment reduce_max over [node, 64|32]; node
  phase L3/L4 + sigmoid -> per-node c. Static u rows ([x0,x1,ea0,ea1] per
  group) and dynamic c rows are separate inputs DMA'd to adjacent partitions.
"""

import time
from contextlib import ExitStack

import numpy as np

N = 100000
E = 3200000
NB = 32            # buckets = 8 cores * 4 streams
NPB_REAL = 3125    # real nodes per bucket
NPB = 3200         # padded nodes per bucket
D = 64             # max slots per node handled on device
CH = 512           # psum chunk
SUPER = 4096       # slots per input dma super-chunk
F32 = np.float32

_cache = {}
LAST_DISPATCH_WALL = None   # total on-device pipeline wall (s) for last call


def _softplus(x):
    return np.logaddexp(0.0, x.astype(np.float64)).astype(F32)


def _sample(wmu, wrho, bmu, brho, eps_w, eps_b):
    W = wmu + _softplus(wrho) * eps_w
    b = bmu + _softplus(brho) * eps_b
    return W.astype(F32), b.astype(F32)


def _build_nc(n_big, with_gather=True):
    import concourse.mybir as mybir
    from concourse.bacc import Bacc
    from concourse.tile import TileContext

    f32 = mybir.dt.float32
    i32 = mybir.dt.int32
    S2 = n_big * 64 + (NPB - n_big) * 32
    XCOL = 4 * S2 // 128          # gather columns (4*S2 slots, 128/instr)
    assert 4 * S2 % 128 == 0 and S2 % XCOL == 0
    nc = Bacc(num_devices=8)
    ustatic = nc.dram_tensor("ustatic", [16, S2], f32, kind="ExternalInput")
    uc = nc.dram_tensor("uc", [4, S2], f32, kind="ExternalInput")
    nstat_s = nc.dram_tensor("nstat_s", [4, 3, NPB], f32, kind="ExternalInput")
    nstat_c = nc.dram_tensor("nstat_c", [4, NPB], f32, kind="ExternalInput")
    if with_gather:
        sidx_pm = nc.dram_tensor(
            "sidx_pm", [128, XCOL], i32, kind="ExternalInput")
    l1w = nc.dram_tensor("l1w", [20, 64], f32, kind="ExternalInput")
    l1b = nc.dram_tensor("l1b", [64, 1], f32, kind="ExternalInput")
    l2w = nc.dram_tensor("l2w", [64, 128], f32, kind="ExternalInput")
    l3w = nc.dram_tensor("l3w", [36, 17], f32, kind="ExternalInput")
    l4w = nc.dram_tensor("l4w", [17, 1], f32, kind="ExternalInput")
    cout = nc.dram_tensor("cout", [4, NPB], f32, kind="ExternalOutput")
    # all-gathered c over all 32 buckets, identical on every core (replicated)
    cgrido = nc.dram_tensor("cgrido", [NB, NPB], f32, kind="ExternalOutput")
    if with_gather:
        # next iteration's per-slot c rows, gathered on device
        ucnext = nc.dram_tensor("ucnext", [4, S2], f32, kind="ExternalOutput")
    cpart = nc.dram_tensor("cpart", [4, NPB], f32)
    cgrid_i = nc.dram_tensor("cgrid_i", [NB, NPB], f32)

    AX = mybir.AxisListType
    OP = mybir.AluOpType
    ACT = mybir.ActivationFunctionType

    with TileContext(nc) as tc, ExitStack() as ctx:
        const = ctx.enter_context(tc.tile_pool(name="const", bufs=1))
        upool = ctx.enter_context(tc.tile_pool(name="u", bufs=2))
        hpool = ctx.enter_context(tc.tile_pool(name="h1", bufs=3))
        p1pool = ctx.enter_context(tc.tile_pool(name="p1", bufs=3, space="PSUM"))
        p2pool = ctx.enter_context(tc.tile_pool(name="p2", bufs=3, space="PSUM"))
        npool = ctx.enter_context(tc.tile_pool(name="np", bufs=1, space="PSUM"))

        # persistent tiles
        l1sb_t = const.tile([84, 64], f32)       # use rows 64:84
        b1t = const.tile([64, 1], f32)
        l2sb_t = const.tile([64, 128], f32)
        l3sb_t = const.tile([36, 17], f32)
        l4sb_t = const.tile([17, 1], f32)
        agg_t = const.tile([128, NPB], f32)
        rhs36_t = const.tile([36, NPB], f32)
        h3_t = const.tile([17, NPB], f32)
        co_t = const.tile([1, 4 * NPB], f32)

        nc.sync.dma_start(l1sb_t[64:84, :], l1w[:, :])
        nc.sync.dma_start(b1t[:, :], l1b[:, :])
        nc.sync.dma_start(l2sb_t[:, :], l2w[:, :])
        nc.sync.dma_start(l3sb_t[:, :], l3w[:, :])
        nc.sync.dma_start(l4sb_t[:, :], l4w[:, :])

        # ---- edge phase ----
        n_super = S2 // SUPER
        for sup in range(n_super):
            u_t = upool.tile([84, SUPER], f32)
            lo = sup * SUPER
            nc.sync.dma_start(u_t[64:80, :], ustatic[:, lo : lo + SUPER])
            nc.sync.dma_start(u_t[80:84, :], uc[:, lo : lo + SUPER])
            for k8 in range(SUPER // CH):
                sl = slice(k8 * CH, (k8 + 1) * CH)
                p1 = p1pool.tile([64, CH], f32)
                nc.tensor.matmul(
                    p1[0:32, :], l1sb_t[64:84, 0:32], u_t[64:84, sl],
                    start=True, stop=True, tile_position=(64, 0),
                )
                nc.tensor.matmul(
                    p1[32:64, :], l1sb_t[64:84, 32:64], u_t[64:84, sl],
                    start=True, stop=True, tile_position=(64, 32),
                )
                h1 = hpool.tile([64, CH], f32)
                nc.scalar.activation(h1[:, :], p1[:, :], ACT.Relu, bias=b1t[:, 0:1])
                p2 = p2pool.tile([128, CH], f32)
                for rg, cg in ((0, 0), (0, 1), (1, 2), (1, 3)):
                    nc.tensor.matmul(
                        p2[32 * cg : 32 * cg + 32, :],
                        l2sb_t[32 * rg : 32 * rg + 32, 32 * cg : 32 * cg + 32],
                        h1[32 * rg : 32 * rg + 32, :],
                        start=True, stop=True, tile_position=(32 * rg, 32 * cg),
                    )
                off = sup * SUPER + k8 * CH
                if off < n_big * 64:
                    d = 64
                    base = off // 64
                else:
                    d = 32
                    base = n_big + (off - n_big * 64) // 32
                nc.vector.tensor_reduce(
                    agg_t[:, base : base + CH // d],
                    p2[:, :].rearrange("p (n d) -> p n d", d=d),
                    axis=AX.X, op=OP.max,
                )

        # ---- node phase ----
        NCH = [(i * CH, min(NPB, (i + 1) * CH)) for i in range((NPB + CH - 1) // CH)]
        for g in range(4):
            nc.sync.dma_start(rhs36_t[0:3, :], nstat_s[g, :, :])
            nc.sync.dma_start(rhs36_t[3:4, :], nstat_c[g : g + 1, :])
            nc.sync.dma_start(rhs36_t[4:36, :], agg_t[32 * g : 32 * g + 32, :])
            for a, b in NCH:
                w = b - a
                pn1 = npool.tile([17, CH], f32, tag="pn1")
                nc.tensor.matmul(
                    pn1[:, :w], l3sb_t[:, :], rhs36_t[:, a:b],
                    start=True, stop=True,
                )
                nc.scalar.activation(h3_t[0:17, a:b], pn1[:, :w], ACT.Relu)
                pn2 = npool.tile([1, CH], f32, tag="pn2")
                nc.tensor.matmul(
                    pn2[:, :w], l4sb_t[:, :], h3_t[:, a:b],
                    start=True, stop=True,
                )
                nc.scalar.activation(
                    co_t[:, g * NPB + a : g * NPB + b], pn2[:, :w], ACT.Sigmoid
                )
            nc.sync.dma_start(cout[g, :], co_t[:, g * NPB : (g + 1) * NPB])
            nc.sync.dma_start(cpart[g, :], co_t[:, g * NPB : (g + 1) * NPB])
        # exchange c across cores so every core carries the full grid
        # (collectives may not write IO tensors -> bounce via internal+SBUF)
        nc.gpsimd.collective_compute(
            "AllGather", mybir.AluOpType.bypass,
            replica_groups=[[i for i in range(8)]],
            ins=[cpart[:, :]], outs=[cgrid_i[:, :]],
        )
        cg_t = const.tile([NB, NPB], f32)
        nc.sync.dma_start(cg_t[:, :], cgrid_i[:, :])
        nc.sync.dma_start(cgrido[:, :], cg_t[:, :])

        if with_gather:
            # ---- on-device gather of next iteration's per-slot c rows ----
            # SWDGE indirect DMA: 128 scattered offsets per instruction (one
            # per dest partition-row). Partition-blocked layout: row p of g_t
            # holds flat slots [p*XCOL, (p+1)*XCOL), so one plain DMA
            # restores the [4, S2] slot-order layout.
            import concourse.bass as _bass
            sidx_t = const.tile([128, XCOL], i32)
            g_t = const.tile([128, XCOL], f32)
            nc.sync.dma_start(sidx_t[:, :], sidx_pm[:, :])
            cflat = cgrid_i.rearrange("a (b o) -> (a b) o", o=1)
            for j in range(XCOL):
                nc.gpsimd.indirect_dma_start(
                    out=g_t[:, j : j + 1], out_offset=None,
                    in_=cflat[:, :],
                    in_offset=_bass.IndirectOffsetOnAxis(
                        ap=sidx_t[:, j : j + 1], axis=0
                    ),
                )
            ucn_pm = ucnext.rearrange("a (x j) -> (a x) j", j=XCOL)
            nc.sync.dma_start(ucn_pm[:, :], g_t[:, :])
    nc.compile()
    return nc


def _preprocess(x, edge_attr, edge_index):
    src = np.asarray(edge_index[0], dtype=np.int64)
    dst = np.asarray(edge_index[1], dtype=np.int64)
    deg = np.bincount(dst, minlength=N)
    order = np.argsort(dst, kind="stable")
    starts = np.zeros(N + 1, dtype=np.int64)
    starts[1:] = np.cumsum(deg)

    # bad nodes: degree 0 (agg=0 path) or degree > D (grid overflow) -> host fix
    bad = np.where((deg == 0) | (deg > D))[0]

    # deal nodes round-robin by degree rank into 32 buckets
    rank = np.argsort(-deg, kind="stable")
    node_bucket = np.empty(N, dtype=np.int64)
    node_pos = np.empty(N, dtype=np.int64)
    node_bucket[rank] = np.arange(N) % NB
    node_pos[rank] = np.arange(N) // NB

    # per-node slot -> edge id (in dst-sorted order), clipped/duplicated
    offs = np.minimum(np.arange(D)[None, :], np.maximum(deg, 1)[:, None] - 1)
    eid = order[np.clip(starts[:N, None] + offs, 0, E - 1)]  # [N, D]

    # bucket node lists (padded with node 0 whose output we ignore)
    nodes_of = np.zeros((NB, NPB), dtype=np.int64)
    nodes_of[node_bucket, node_pos] = np.arange(N)
    # two-class padding: first n_big positions (degree-sorted desc within each
    # bucket) get 64 slots, the rest 32, rounded to 128 for chunk alignment.
    big_counts = np.bincount(node_bucket[deg > 32], minlength=NB)
    n_big = int(big_counts.max())
    n_big = min(NPB, ((n_big + 127) // 128) * 128)
    eid_big = eid[nodes_of[:, :n_big].reshape(-1)].reshape(NB, n_big * 64)
    eid_small = eid[nodes_of[:, n_big:].reshape(-1)].reshape(NB, NPB - n_big, D)
    slot_eid = np.concatenate(
        [eid_big, eid_small[:, :, :32].reshape(NB, -1)], axis=1
    )
    slot_src = src[slot_eid]
    # grid index (bucket*NPB + pos) of each slot's src node, for device gather
    gidx = node_bucket * NPB + node_pos
    sidx = gidx[slot_src].astype(np.int32)      # [NB, S2]
    return dict(
        src=src, dst=dst, deg=deg, order=order, starts=starts, bad=bad,
        nodes_of=nodes_of, slot_src=slot_src, slot_eid=slot_eid, sidx=sidx,
        node_bucket=node_bucket, node_pos=node_pos, n_big=n_big,
    )


def _host_fix(c_prev, c_new, pp, params, x2, edge_attr):
    """Exact numpy IGConv for bad nodes (deg 0 or > D)."""
    (W1, b1), (W2, b2), (W3, b3), (W4, b4) = params
    src, deg, order, starts = pp["src"], pp["deg"], pp["order"], pp["starts"]
    for n in pp["bad"]:
        d = deg[n]
        if d == 0:
            agg = np.zeros(32, dtype=F32)
        else:
            es = order[starts[n] : starts[n] + d]
            u = np.concatenate(
                [x2[src[es]], c_prev[src[es]][:, None], edge_attr[es]], axis=1
            )  # [d, 5] with order [x0,x1,c,ea0,ea1]
            h1 = np.maximum(u @ W1.T + b1, 0.0)
            m = h1 @ W2.T + b2
            agg = m.max(axis=0)
        h = np.concatenate([x2[n], [c_prev[n]], agg]).astype(F32)
        h3 = np.maximum(h @ W3.T + b3, 0.0)
        c_new[n] = 1.0 / (1.0 + np.exp(-(h3 @ W4.T + b4)))[0]
    return c_new


class _Dispatcher:
    """Cached jitted shard_map dispatch of a Bass module over 8 cores,
    mirroring bass2jax.run_bass_via_pjrt but reusing the compiled fn and
    accepting device-resident jax arrays."""

    def __init__(self, nc):
        import jax
        import concourse.mybir as mybir
        from concourse.bass2jax import (
            _bass_exec_p, install_neuronx_cc_hook, partition_id_tensor,
        )
        from jax.experimental.shard_map import shard_map
        from jax.sharding import Mesh, PartitionSpec, NamedSharding

        install_neuronx_cc_hook()
        self.nc = nc
        self.n_cores = 8
        partition_name = (
            nc.partition_id_tensor.name if nc.partition_id_tensor else None
        )
        in_names, out_names, out_avals, zero_outs = [], [], [], []
        for alloc in nc.m.functions[0].allocations:
            if not isinstance(alloc, mybir.MemoryLocationSet):
                continue
            name = alloc.memorylocations[0].name
            if alloc.kind == "ExternalInput":
                if name != partition_name:
                    in_names.append(name)
            elif alloc.kind == "ExternalOutput":
                shape = tuple(alloc.tensor_shape)
                dtype = mybir.dt.np(alloc.dtype)
                out_names.append(name)
                out_avals.append(jax.core.ShapedArray(shape, dtype))
                zero_outs.append(np.zeros(shape, dtype))
        self.in_names = in_names
        self.out_names = out_names
        self.zero_outs = zero_outs
        n_params = len(in_names)
        n_outs = len(out_avals)
        in_names_all = in_names + out_names
        if partition_name is not None:
            in_names_all = in_names_all + [partition_name]
        donate = tuple(range(n_params, n_params + n_outs))

        def _body(*args):
            operands = list(args)
            if partition_name is not None:
                operands.append(partition_id_tensor())
            outs = _bass_exec_p.bind(
                *operands, out_avals=tuple(out_avals),
                in_names=tuple(in_names_all), out_names=tuple(out_names),
                lowering_input_output_aliases=(),
                sim_require_finite=True, sim_require_nnan=True, nc=nc)
            return tuple(outs)

        devices = jax.devices()[: self.n_cores]
        self.mesh = Mesh(np.asarray(devices), ("core",))
        self.sh_core = NamedSharding(self.mesh, PartitionSpec("core"))
        self.sh_rep = NamedSharding(self.mesh, PartitionSpec())
        # outputs identical on every core (written by an in-kernel AllGather)
        # are exposed replicated instead of core-sharded
        rep_outs = {"cgrido"}
        out_pspecs = [
            PartitionSpec() if nm in rep_outs else PartitionSpec("core")
            for nm in out_names
        ]
        in_specs = (PartitionSpec("core"),) * n_params + tuple(out_pspecs)
        self.fn = jax.jit(
            shard_map(_body, mesh=self.mesh, in_specs=in_specs,
                      out_specs=tuple(out_pspecs), check_rep=False),
            donate_argnums=donate, keep_unused=True)

        import jax.numpy as jnp
        zshapes = [
            ((z.shape if nm in rep_outs
              else (self.n_cores * z.shape[0], *z.shape[1:])), z.dtype)
            for nm, z in zip(out_names, self.zero_outs)
        ]
        zshard = tuple(
            self.sh_rep if nm in rep_outs else self.sh_core for nm in out_names
        )
        self._zeros_fn = jax.jit(
            lambda: tuple(jnp.zeros(s, d) for s, d in zshapes),
            out_shardings=zshard)

    def __call__(self, arrays_by_name):
        args = [arrays_by_name[nm] for nm in self.in_names]
        zeros = self._zeros_fn()   # device-side zeros (donated per dispatch)
        outs = self.fn(*args, *zeros)
        return dict(zip(self.out_names, outs))


def _get_ctx(x, edge_attr, edge_index):
    """Build (once) preprocessing + device-resident static arrays + jits."""
    if "ctx" in _cache:
        return _cache["ctx"]
    import jax
    import jax.numpy as jnp

    x = np.asarray(x, dtype=F32)
    edge_attr = np.asarray(edge_attr, dtype=F32)
    pp = _preprocess(x, edge_attr, np.asarray(edge_index))
    S2 = pp["slot_src"].shape[1]
    slot_src = pp["slot_src"]
    nodes_of = pp["nodes_of"]

    # static per-slot inputs: ustatic rows 4g+k = [x0,x1,ea0,ea1] of bucket
    # 4cc+g; uc0 row g = initial c (= x[:,2]) per slot.
    ea = edge_attr[pp["slot_eid"]]          # [NB, S2, 2]
    ustatic = np.empty((8, 16, S2), dtype=F32)
    uc0 = np.empty((8, 4, S2), dtype=F32)
    c0 = x[:, 2].astype(F32)
    for b in range(NB):
        cc, g = divmod(b, 4)
        ustatic[cc, 4 * g + 0] = x[:, 0][slot_src[b]]
        ustatic[cc, 4 * g + 1] = x[:, 1][slot_src[b]]
        ustatic[cc, 4 * g + 2] = ea[b, :, 0]
        ustatic[cc, 4 * g + 3] = ea[b, :, 1]
        uc0[cc, g] = c0[slot_src[b]]

    nstat_s = np.empty((NB, 3, NPB), dtype=F32)
    nstat_s[:, 0, :] = x[:, 0][nodes_of]
    nstat_s[:, 1, :] = x[:, 1][nodes_of]
    nstat_s[:, 2, :] = 1.0
    nstat_c0 = c0[nodes_of].astype(F32)      # [NB, NPB]

    nc = _build_nc(pp["n_big"], with_gather=True)
    disp = _Dispatcher(nc)
    nc_ng = _build_nc(pp["n_big"], with_gather=False)
    disp_ng = _Dispatcher(nc_ng)
    sh = disp.sh_core

    # partition-blocked gather index layout: per core, flat [4*S2] slot order
    # reshaped to [128, XCOL] (row p = flat slots [p*XCOL, (p+1)*XCOL))
    XCOL = 4 * S2 // 128
    sidx_pm = pp["sidx"].reshape(8, 4 * S2).reshape(8, 128, XCOL)

    dev = {
        "ustatic": jax.device_put(ustatic.reshape(8 * 16, S2), sh),
        "uc0": jax.device_put(uc0.reshape(8 * 4, S2), sh),
        "nstat_s": jax.device_put(nstat_s.reshape(32, 3, NPB), sh),
        "nstat_c0": jax.device_put(nstat_c0, sh),
        "sidx_pm": jax.device_put(
            np.ascontiguousarray(sidx_pm.reshape(8 * 128, XCOL)), sh),
    }
    jax.block_until_ready(list(dev.values()))

    # device-side x8 tiling of the per-call stationaries (uploads ~40KB of
    # single-core arrays instead of ~325KB pre-tiled)
    import jax.numpy as jnp
    wtile = jax.jit(
        lambda *ws: tuple(jnp.tile(w, (8,) + (1,) * (w.ndim - 1)) for w in ws),
        out_shardings=(sh,) * 5)

    ctx = dict(pp=pp, nc=nc, disp=disp, disp_ng=disp_ng, dev=dev, S2=S2,
               wtile=wtile, x2=x[:, :2].copy(), edge_attr=edge_attr)
    _cache["ctx"] = ctx
    return ctx


def _stationaries(params):
    (W1, b1), (W2, b2), (W3, b3), (W4, b4) = params
    # L1 stationary row order: rows 4g+k = group g inputs [x0,x1,ea0,ea1],
    # rows 16+g = group g input c.
    l1blk = np.zeros((20, 64), dtype=F32)
    l2blk = np.zeros((64, 128), dtype=F32)
    for g in range(4):
        for k, col in enumerate((0, 1, 3, 4)):
            l1blk[4 * g + k, 16 * g : 16 * g + 16] = W1[:, col]
        l1blk[16 + g, 16 * g : 16 * g + 16] = W1[:, 2]
        l2blk[16 * g : 16 * g + 16, 32 * g : 32 * g + 32] = W2.T
    l1bv = np.ascontiguousarray(np.tile(b1, 4)[:, None]).astype(F32)  # [64,1]
    b3p = b3 + W3[:, 3:] @ b2
    # rhs36 row order: x0, x1, ones, c, agg[0:32]
    l3w = np.zeros((36, 17), dtype=F32)
    l3w[0:2, :16] = W3[:, 0:2].T    # x0, x1
    l3w[2, :16] = b3p               # ones row carries bias
    l3w[3, :16] = W3[:, 2]          # c
    l3w[4:36, :16] = W3[:, 3:].T    # agg
    l3w[2, 16] = 1.0                # emits constant 1 -> h3 row 16
    l4w = np.concatenate([W4.T, b4[None, :]], axis=0).astype(F32)  # [17, 1]
    return l1blk, l1bv, l2blk, l3w, l4w


def kernel(**inputs):
    import jax
    global LAST_DISPATCH_WALL

    x = np.asarray(inputs["x"], dtype=F32)
    edge_attr = np.asarray(inputs["edge_attr"], dtype=F32)
    ctx = _get_ctx(x, edge_attr, inputs["edge_index"])
    pp, disp, dev = ctx["pp"], ctx["disp"], ctx["dev"]

    params = []
    for li in (1, 2, 3, 4):
        params.append(
            _sample(
                inputs[f"l{li}_wmu"], inputs[f"l{li}_wrho"],
                inputs[f"l{li}_bmu"], inputs[f"l{li}_brho"],
                inputs[f"l{li}_eps_w"], inputs[f"l{li}_eps_b"],
            )
        )
    l1blk, l1bv, l2blk, l3w, l4w = _stationaries(params)
    wt = ctx["wtile"](l1blk, l1bv, l2blk, l3w, l4w)
    weights = dict(zip(("l1w", "l1b", "l2w", "l3w", "l4w"), wt))

    slow_path = pp["bad"].size > 0
    nodes_of = pp["nodes_of"]
    t0 = time.time()
    if not slow_path:
        args = dict(weights)
        args["ustatic"] = dev["ustatic"]
        args["nstat_s"] = dev["nstat_s"]
        args["sidx_pm"] = dev["sidx_pm"]
        args["uc"] = dev["uc0"]
        args["nstat_c"] = dev["nstat_c0"]
        for it in range(3):
            # last iteration uses the gather-free NEFF variant
            outs = (disp if it < 2 else ctx["disp_ng"])(args)
            if it < 2:
                # uc for the next iteration was gathered on device
                args["uc"] = outs["ucnext"]
                args["nstat_c"] = outs["cout"]
        cgrid = np.asarray(outs["cgrido"])     # fetch [32, NPB]
        LAST_DISPATCH_WALL = time.time() - t0
        c = np.empty(N, dtype=F32)
        c[nodes_of.reshape(-1)] = cgrid.reshape(-1)
        # pad positions point at node 0; restore its own value
        b0 = int(pp["node_bucket"][0]); p0 = int(pp["node_pos"][0])
        c[0] = cgrid[b0, p0]
    else:
        # correctness fallback: host roundtrip each iteration to fix bad nodes
        c = x[:, 2].astype(F32).copy()
        slot_src = pp["slot_src"]
        S2 = ctx["S2"]
        for _ in range(3):
            uc = np.empty((8, 4, S2), dtype=F32)
            for b in range(NB):
                cc, g = divmod(b, 4)
                uc[cc, g] = c[slot_src[b]]
            args = dict(weights)
            args["ustatic"] = dev["ustatic"]
            args["nstat_s"] = dev["nstat_s"]
            args["uc"] = uc.reshape(32, S2)
            args["nstat_c"] = c[nodes_of].astype(F32)
            cgrid = np.asarray(ctx["disp_ng"](args)["cout"])
            c_new = np.empty(N, dtype=F32)
            c_new[nodes_of.reshape(-1)] = cgrid.reshape(-1)
            b0 = int(pp["node_bucket"][0]); p0 = int(pp["node_pos"][0])
            c_new[0] = cgrid[b0, p0]
            c_new = _host_fix(c, c_new, pp, params, ctx["x2"], edge_attr)
            c = c_new
        LAST_DISPATCH_WALL = time.time() - t0

    out = np.empty((N, 3), dtype=F32)
    out[:, :2] = x[:, :2]
    out[:, 2] = c
    return out


# revision 27
# speedup vs baseline: 1.8196x; 1.8196x over previous
"""BayesianIGCNet Trainium2 kernel.

Strategy (node-sharded dst-grid, device-resident statics, fully on-device
iteration chain — 3 Bass dispatches per call, no host roundtrips between
iterations):
- Host (once, cached): sort edges by dst, deal nodes round-robin by degree
  rank into 32 buckets (8 cores x 4 streams), two-class pad each node's slot
  run (deg<=32 -> 32 slots, else 64) by duplicating a real edge (max is
  idempotent), precompute static per-slot L1 inputs and partition-blocked
  src-grid gather indices; device_put all static arrays once (sharded over
  the 8 cores).
- Per call: upload only the sampled single-core stationaries (~40KB, tiled
  x8 on device). Run 3 chained Bass dispatches whose intermediates stay on
  device, then fetch the final [32,3200] c grid (replicated output, one
  410KB read).
- Bass NEFF (one IGConv layer): L1/L2 matmuls with persistent block-diagonal
  stationaries, ACT relu, DVE segment reduce_max over [node, 64|32]; node
  phase L3/L4 + sigmoid -> per-node c. Then an AllGather collective
  exchanges each core's [4,3200] c block into the full replicated grid, and
  4736 unrolled SWDGE indirect DMAs ([128,1] dest = 128 scattered offsets
  per instruction) gather the next iteration's per-slot c rows on device
  into a ucnext output that feeds the next dispatch directly. The last
  dispatch uses a gather-free NEFF variant.
"""

import time
from contextlib import ExitStack

import numpy as np

N = 100000
E = 3200000
NB = 32            # buckets = 8 cores * 4 streams
NPB_REAL = 3125    # real nodes per bucket
NPB = 3200         # padded nodes per bucket
D = 64             # max slots per node handled on device
CH = 512           # psum chunk
SUPER = 4096       # slots per input dma super-chunk
F32 = np.float32

_cache = {}
LAST_DISPATCH_WALL = None   # total on-device pipeline wall (s) for last call


def _softplus(x):
    return np.logaddexp(0.0, x.astype(np.float64)).astype(F32)


def _sample(wmu, wrho, bmu, brho, eps_w, eps_b):
    W = wmu + _softplus(wrho) * eps_w
    b = bmu + _softplus(brho) * eps_b
    return W.astype(F32), b.astype(F32)


def _build_nc(n_big, with_gather=True):
    import concourse.mybir as mybir
    from concourse.bacc import Bacc
    from concourse.tile import TileContext

    f32 = mybir.dt.float32
    i32 = mybir.dt.int32
    S2 = n_big * 64 + (NPB - n_big) * 32
    XCOL = 4 * S2 // 128          # gather columns (4*S2 slots, 128/instr)
    assert 4 * S2 % 128 == 0 and S2 % XCOL == 0
    nc = Bacc(num_devices=8)
    ustatic = nc.dram_tensor("ustatic", [16, S2], f32, kind="ExternalInput")
    uc = nc.dram_tensor("uc", [4, S2], f32, kind="ExternalInput")
    nstat_s = nc.dram_tensor("nstat_s", [4, 3, NPB], f32, kind="ExternalInput")
    nstat_c = nc.dram_tensor("nstat_c", [4, NPB], f32, kind="ExternalInput")
    if with_gather:
        sidx_pm = nc.dram_tensor(
            "sidx_pm", [128, XCOL], i32, kind="ExternalInput")
    l1w = nc.dram_tensor("l1w", [20, 64], f32, kind="ExternalInput")
    l1b = nc.dram_tensor("l1b", [64, 1], f32, kind="ExternalInput")
    l2w = nc.dram_tensor("l2w", [64, 128], f32, kind="ExternalInput")
    l3w = nc.dram_tensor("l3w", [36, 17], f32, kind="ExternalInput")
    l4w = nc.dram_tensor("l4w", [17, 1], f32, kind="ExternalInput")
    cout = nc.dram_tensor("cout", [4, NPB], f32, kind="ExternalOutput")
    # all-gathered c over all 32 buckets, identical on every core (replicated)
    cgrido = nc.dram_tensor("cgrido", [NB, NPB], f32, kind="ExternalOutput")
    if with_gather:
        # next iteration's per-slot c rows, gathered on device
        ucnext = nc.dram_tensor("ucnext", [4, S2], f32, kind="ExternalOutput")
    cpart = nc.dram_tensor("cpart", [4, NPB], f32)
    cgrid_i = nc.dram_tensor("cgrid_i", [NB, NPB], f32)

    AX = mybir.AxisListType
    OP = mybir.AluOpType
    ACT = mybir.ActivationFunctionType

    with TileContext(nc) as tc, ExitStack() as ctx:
        const = ctx.enter_context(tc.tile_pool(name="const", bufs=1))
        upool = ctx.enter_context(tc.tile_pool(name="u", bufs=2))
        hpool = ctx.enter_context(tc.tile_pool(name="h1", bufs=3))
        p1pool = ctx.enter_context(tc.tile_pool(name="p1", bufs=3, space="PSUM"))
        p2pool = ctx.enter_context(tc.tile_pool(name="p2", bufs=3, space="PSUM"))
        npool = ctx.enter_context(tc.tile_pool(name="np", bufs=1, space="PSUM"))

        # persistent tiles
        l1sb_t = const.tile([84, 64], f32)       # use rows 64:84
        b1t = const.tile([64, 1], f32)
        l2sb_t = const.tile([64, 128], f32)
        l3sb_t = const.tile([36, 17], f32)
        l4sb_t = const.tile([17, 1], f32)
        agg_t = const.tile([128, NPB], f32)
        rhs36_t = const.tile([36, NPB], f32)
        h3_t = const.tile([17, NPB], f32)
        co_t = const.tile([1, 4 * NPB], f32)

        nc.sync.dma_start(l1sb_t[64:84, :], l1w[:, :])
        nc.sync.dma_start(b1t[:, :], l1b[:, :])
        nc.sync.dma_start(l2sb_t[:, :], l2w[:, :])
        nc.sync.dma_start(l3sb_t[:, :], l3w[:, :])
        nc.sync.dma_start(l4sb_t[:, :], l4w[:, :])

        # ---- edge phase ----
        n_super = S2 // SUPER
        for sup in range(n_super):
            u_t = upool.tile([84, SUPER], f32)
            lo = sup * SUPER
            nc.sync.dma_start(u_t[64:80, :], ustatic[:, lo : lo + SUPER])
            nc.sync.dma_start(u_t[80:84, :], uc[:, lo : lo + SUPER])
            for k8 in range(SUPER // CH):
                sl = slice(k8 * CH, (k8 + 1) * CH)
                p1 = p1pool.tile([64, CH], f32)
                nc.tensor.matmul(
                    p1[0:32, :], l1sb_t[64:84, 0:32], u_t[64:84, sl],
                    start=True, stop=True, tile_position=(64, 0),
                )
                nc.tensor.matmul(
                    p1[32:64, :], l1sb_t[64:84, 32:64], u_t[64:84, sl],
                    start=True, stop=True, tile_position=(64, 32),
                )
                h1 = hpool.tile([64, CH], f32)
                nc.scalar.activation(h1[:, :], p1[:, :], ACT.Relu, bias=b1t[:, 0:1])
                p2 = p2pool.tile([128, CH], f32)
                for rg, cg in ((0, 0), (0, 1), (1, 2), (1, 3)):
                    nc.tensor.matmul(
                        p2[32 * cg : 32 * cg + 32, :],
                        l2sb_t[32 * rg : 32 * rg + 32, 32 * cg : 32 * cg + 32],
                        h1[32 * rg : 32 * rg + 32, :],
                        start=True, stop=True, tile_position=(32 * rg, 32 * cg),
                    )
                off = sup * SUPER + k8 * CH
                if off < n_big * 64:
                    d = 64
                    base = off // 64
                else:
                    d = 32
                    base = n_big + (off - n_big * 64) // 32
                nc.vector.tensor_reduce(
                    agg_t[:, base : base + CH // d],
                    p2[:, :].rearrange("p (n d) -> p n d", d=d),
                    axis=AX.X, op=OP.max,
                )

        # ---- node phase ----
        NCH = [(i * CH, min(NPB, (i + 1) * CH)) for i in range((NPB + CH - 1) // CH)]
        for g in range(4):
            nc.sync.dma_start(rhs36_t[0:3, :], nstat_s[g, :, :])
            nc.sync.dma_start(rhs36_t[3:4, :], nstat_c[g : g + 1, :])
            nc.sync.dma_start(rhs36_t[4:36, :], agg_t[32 * g : 32 * g + 32, :])
            for a, b in NCH:
                w = b - a
                pn1 = npool.tile([17, CH], f32, tag="pn1")
                nc.tensor.matmul(
                    pn1[:, :w], l3sb_t[:, :], rhs36_t[:, a:b],
                    start=True, stop=True,
                )
                nc.scalar.activation(h3_t[0:17, a:b], pn1[:, :w], ACT.Relu)
                pn2 = npool.tile([1, CH], f32, tag="pn2")
                nc.tensor.matmul(
                    pn2[:, :w], l4sb_t[:, :], h3_t[:, a:b],
                    start=True, stop=True,
                )
                nc.scalar.activation(
                    co_t[:, g * NPB + a : g * NPB + b], pn2[:, :w], ACT.Sigmoid
                )
            nc.sync.dma_start(cout[g, :], co_t[:, g * NPB : (g + 1) * NPB])
            nc.sync.dma_start(cpart[g, :], co_t[:, g * NPB : (g + 1) * NPB])
        # exchange c across cores so every core carries the full grid
        # (collectives may not write IO tensors -> bounce via internal+SBUF)
        nc.gpsimd.collective_compute(
            "AllGather", mybir.AluOpType.bypass,
            replica_groups=[[i for i in range(8)]],
            ins=[cpart[:, :]], outs=[cgrid_i[:, :]],
        )
        cg_t = const.tile([NB, NPB], f32)
        nc.sync.dma_start(cg_t[:, :], cgrid_i[:, :])
        nc.sync.dma_start(cgrido[:, :], cg_t[:, :])

        if with_gather:
            # ---- on-device gather of next iteration's per-slot c rows ----
            # SWDGE indirect DMA: 128 scattered offsets per instruction (one
            # per dest partition-row). Partition-blocked layout: row p of g_t
            # holds flat slots [p*XCOL, (p+1)*XCOL), so one plain DMA
            # restores the [4, S2] slot-order layout.
            import concourse.bass as _bass
            sidx_t = const.tile([128, XCOL], i32)
            g_t = const.tile([128, XCOL], f32)
            nc.sync.dma_start(sidx_t[:, :], sidx_pm[:, :])
            cflat = cgrid_i.rearrange("a (b o) -> (a b) o", o=1)
            for j in range(XCOL):
                nc.gpsimd.indirect_dma_start(
                    out=g_t[:, j : j + 1], out_offset=None,
                    in_=cflat[:, :],
                    in_offset=_bass.IndirectOffsetOnAxis(
                        ap=sidx_t[:, j : j + 1], axis=0
                    ),
                )
            ucn_pm = ucnext.rearrange("a (x j) -> (a x) j", j=XCOL)
            nc.sync.dma_start(ucn_pm[:, :], g_t[:, :])
    nc.compile()
    return nc


def _preprocess(x, edge_attr, edge_index):
    src = np.asarray(edge_index[0], dtype=np.int64)
    dst = np.asarray(edge_index[1], dtype=np.int64)
    deg = np.bincount(dst, minlength=N)
    order = np.argsort(dst, kind="stable")
    starts = np.zeros(N + 1, dtype=np.int64)
    starts[1:] = np.cumsum(deg)

    # bad nodes: degree 0 (agg=0 path) or degree > D (grid overflow) -> host fix
    bad = np.where((deg == 0) | (deg > D))[0]

    # deal nodes round-robin by degree rank into 32 buckets
    rank = np.argsort(-deg, kind="stable")
    node_bucket = np.empty(N, dtype=np.int64)
    node_pos = np.empty(N, dtype=np.int64)
    node_bucket[rank] = np.arange(N) % NB
    node_pos[rank] = np.arange(N) // NB

    # per-node slot -> edge id (in dst-sorted order), clipped/duplicated
    offs = np.minimum(np.arange(D)[None, :], np.maximum(deg, 1)[:, None] - 1)
    eid = order[np.clip(starts[:N, None] + offs, 0, E - 1)]  # [N, D]

    # bucket node lists (padded with node 0 whose output we ignore)
    nodes_of = np.zeros((NB, NPB), dtype=np.int64)
    nodes_of[node_bucket, node_pos] = np.arange(N)
    # two-class padding: first n_big positions (degree-sorted desc within each
    # bucket) get 64 slots, the rest 32, rounded to 128 for chunk alignment.
    big_counts = np.bincount(node_bucket[deg > 32], minlength=NB)
    n_big = int(big_counts.max())
    n_big = min(NPB, ((n_big + 127) // 128) * 128)
    eid_big = eid[nodes_of[:, :n_big].reshape(-1)].reshape(NB, n_big * 64)
    eid_small = eid[nodes_of[:, n_big:].reshape(-1)].reshape(NB, NPB - n_big, D)
    slot_eid = np.concatenate(
        [eid_big, eid_small[:, :, :32].reshape(NB, -1)], axis=1
    )
    slot_src = src[slot_eid]
    # grid index (bucket*NPB + pos) of each slot's src node, for device gather
    gidx = node_bucket * NPB + node_pos
    sidx = gidx[slot_src].astype(np.int32)      # [NB, S2]
    return dict(
        src=src, dst=dst, deg=deg, order=order, starts=starts, bad=bad,
        nodes_of=nodes_of, slot_src=slot_src, slot_eid=slot_eid, sidx=sidx,
        node_bucket=node_bucket, node_pos=node_pos, n_big=n_big,
    )


def _host_fix(c_prev, c_new, pp, params, x2, edge_attr):
    """Exact numpy IGConv for bad nodes (deg 0 or > D)."""
    (W1, b1), (W2, b2), (W3, b3), (W4, b4) = params
    src, deg, order, starts = pp["src"], pp["deg"], pp["order"], pp["starts"]
    for n in pp["bad"]:
        d = deg[n]
        if d == 0:
            agg = np.zeros(32, dtype=F32)
        else:
            es = order[starts[n] : starts[n] + d]
            u = np.concatenate(
                [x2[src[es]], c_prev[src[es]][:, None], edge_attr[es]], axis=1
            )  # [d, 5] with order [x0,x1,c,ea0,ea1]
            h1 = np.maximum(u @ W1.T + b1, 0.0)
            m = h1 @ W2.T + b2
            agg = m.max(axis=0)
        h = np.concatenate([x2[n], [c_prev[n]], agg]).astype(F32)
        h3 = np.maximum(h @ W3.T + b3, 0.0)
        c_new[n] = 1.0 / (1.0 + np.exp(-(h3 @ W4.T + b4)))[0]
    return c_new


class _Dispatcher:
    """Cached jitted shard_map dispatch of a Bass module over 8 cores,
    mirroring bass2jax.run_bass_via_pjrt but reusing the compiled fn and
    accepting device-resident jax arrays."""

    def __init__(self, nc):
        import jax
        import concourse.mybir as mybir
        from concourse.bass2jax import (
            _bass_exec_p, install_neuronx_cc_hook, partition_id_tensor,
        )
        from jax.experimental.shard_map import shard_map
        from jax.sharding import Mesh, PartitionSpec, NamedSharding

        install_neuronx_cc_hook()
        self.nc = nc
        self.n_cores = 8
        partition_name = (
            nc.partition_id_tensor.name if nc.partition_id_tensor else None
        )
        in_names, out_names, out_avals, zero_outs = [], [], [], []
        for alloc in nc.m.functions[0].allocations:
            if not isinstance(alloc, mybir.MemoryLocationSet):
                continue
            name = alloc.memorylocations[0].name
            if alloc.kind == "ExternalInput":
                if name != partition_name:
                    in_names.append(name)
            elif alloc.kind == "ExternalOutput":
                shape = tuple(alloc.tensor_shape)
                dtype = mybir.dt.np(alloc.dtype)
                out_names.append(name)
                out_avals.append(jax.core.ShapedArray(shape, dtype))
                zero_outs.append(np.zeros(shape, dtype))
        self.in_names = in_names
        self.out_names = out_names
        self.zero_outs = zero_outs
        n_params = len(in_names)
        n_outs = len(out_avals)
        in_names_all = in_names + out_names
        if partition_name is not None:
            in_names_all = in_names_all + [partition_name]
        donate = tuple(range(n_params, n_params + n_outs))

        def _body(*args):
            operands = list(args)
            if partition_name is not None:
                operands.append(partition_id_tensor())
            outs = _bass_exec_p.bind(
                *operands, out_avals=tuple(out_avals),
                in_names=tuple(in_names_all), out_names=tuple(out_names),
                lowering_input_output_aliases=(),
                sim_require_finite=True, sim_require_nnan=True, nc=nc)
            return tuple(outs)

        devices = jax.devices()[: self.n_cores]
        self.mesh = Mesh(np.asarray(devices), ("core",))
        self.sh_core = NamedSharding(self.mesh, PartitionSpec("core"))
        self.sh_rep = NamedSharding(self.mesh, PartitionSpec())
        # outputs identical on every core (written by an in-kernel AllGather)
        # are exposed replicated instead of core-sharded
        rep_outs = {"cgrido"}
        out_pspecs = [
            PartitionSpec() if nm in rep_outs else PartitionSpec("core")
            for nm in out_names
        ]
        in_specs = (PartitionSpec("core"),) * n_params + tuple(out_pspecs)
        self.fn = jax.jit(
            shard_map(_body, mesh=self.mesh, in_specs=in_specs,
                      out_specs=tuple(out_pspecs), check_rep=False),
            donate_argnums=donate, keep_unused=True)

        import jax.numpy as jnp
        zshapes = [
            ((z.shape if nm in rep_outs
              else (self.n_cores * z.shape[0], *z.shape[1:])), z.dtype)
            for nm, z in zip(out_names, self.zero_outs)
        ]
        zshard = tuple(
            self.sh_rep if nm in rep_outs else self.sh_core for nm in out_names
        )
        self._zeros_fn = jax.jit(
            lambda: tuple(jnp.zeros(s, d) for s, d in zshapes),
            out_shardings=zshard)

    def __call__(self, arrays_by_name):
        args = [arrays_by_name[nm] for nm in self.in_names]
        zeros = self._zeros_fn()   # device-side zeros (donated per dispatch)
        outs = self.fn(*args, *zeros)
        return dict(zip(self.out_names, outs))


def _get_ctx(x, edge_attr, edge_index):
    """Build (once) preprocessing + device-resident static arrays + jits."""
    if "ctx" in _cache:
        return _cache["ctx"]
    import jax
    import jax.numpy as jnp

    x = np.asarray(x, dtype=F32)
    edge_attr = np.asarray(edge_attr, dtype=F32)
    pp = _preprocess(x, edge_attr, np.asarray(edge_index))
    S2 = pp["slot_src"].shape[1]
    slot_src = pp["slot_src"]
    nodes_of = pp["nodes_of"]

    # static per-slot inputs: ustatic rows 4g+k = [x0,x1,ea0,ea1] of bucket
    # 4cc+g; uc0 row g = initial c (= x[:,2]) per slot.
    ea = edge_attr[pp["slot_eid"]]          # [NB, S2, 2]
    ustatic = np.empty((8, 16, S2), dtype=F32)
    uc0 = np.empty((8, 4, S2), dtype=F32)
    c0 = x[:, 2].astype(F32)
    for b in range(NB):
        cc, g = divmod(b, 4)
        ustatic[cc, 4 * g + 0] = x[:, 0][slot_src[b]]
        ustatic[cc, 4 * g + 1] = x[:, 1][slot_src[b]]
        ustatic[cc, 4 * g + 2] = ea[b, :, 0]
        ustatic[cc, 4 * g + 3] = ea[b, :, 1]
        uc0[cc, g] = c0[slot_src[b]]

    nstat_s = np.empty((NB, 3, NPB), dtype=F32)
    nstat_s[:, 0, :] = x[:, 0][nodes_of]
    nstat_s[:, 1, :] = x[:, 1][nodes_of]
    nstat_s[:, 2, :] = 1.0
    nstat_c0 = c0[nodes_of].astype(F32)      # [NB, NPB]

    nc = _build_nc(pp["n_big"], with_gather=True)
    disp = _Dispatcher(nc)
    nc_ng = _build_nc(pp["n_big"], with_gather=False)
    disp_ng = _Dispatcher(nc_ng)
    sh = disp.sh_core

    # partition-blocked gather index layout: per core, flat [4*S2] slot order
    # reshaped to [128, XCOL] (row p = flat slots [p*XCOL, (p+1)*XCOL))
    XCOL = 4 * S2 // 128
    sidx_pm = pp["sidx"].reshape(8, 4 * S2).reshape(8, 128, XCOL)

    dev = {
        "ustatic": jax.device_put(ustatic.reshape(8 * 16, S2), sh),
        "uc0": jax.device_put(uc0.reshape(8 * 4, S2), sh),
        "nstat_s": jax.device_put(nstat_s.reshape(32, 3, NPB), sh),
        "nstat_c0": jax.device_put(nstat_c0, sh),
        "sidx_pm": jax.device_put(
            np.ascontiguousarray(sidx_pm.reshape(8 * 128, XCOL)), sh),
    }
    jax.block_until_ready(list(dev.values()))

    # device-side x8 tiling of the per-call stationaries (uploads ~40KB of
    # single-core arrays instead of ~325KB pre-tiled)
    import jax.numpy as jnp
    wtile = jax.jit(
        lambda *ws: tuple(jnp.tile(w, (8,) + (1,) * (w.ndim - 1)) for w in ws),
        out_shardings=(sh,) * 5)

    ctx = dict(pp=pp, nc=nc, disp=disp, disp_ng=disp_ng, dev=dev, S2=S2,
               wtile=wtile, x2=x[:, :2].copy(), edge_attr=edge_attr)
    _cache["ctx"] = ctx
    return ctx


def _stationaries(params):
    (W1, b1), (W2, b2), (W3, b3), (W4, b4) = params
    # L1 stationary row order: rows 4g+k = group g inputs [x0,x1,ea0,ea1],
    # rows 16+g = group g input c.
    l1blk = np.zeros((20, 64), dtype=F32)
    l2blk = np.zeros((64, 128), dtype=F32)
    for g in range(4):
        for k, col in enumerate((0, 1, 3, 4)):
            l1blk[4 * g + k, 16 * g : 16 * g + 16] = W1[:, col]
        l1blk[16 + g, 16 * g : 16 * g + 16] = W1[:, 2]
        l2blk[16 * g : 16 * g + 16, 32 * g : 32 * g + 32] = W2.T
    l1bv = np.ascontiguousarray(np.tile(b1, 4)[:, None]).astype(F32)  # [64,1]
    b3p = b3 + W3[:, 3:] @ b2
    # rhs36 row order: x0, x1, ones, c, agg[0:32]
    l3w = np.zeros((36, 17), dtype=F32)
    l3w[0:2, :16] = W3[:, 0:2].T    # x0, x1
    l3w[2, :16] = b3p               # ones row carries bias
    l3w[3, :16] = W3[:, 2]          # c
    l3w[4:36, :16] = W3[:, 3:].T    # agg
    l3w[2, 16] = 1.0                # emits constant 1 -> h3 row 16
    l4w = np.concatenate([W4.T, b4[None, :]], axis=0).astype(F32)  # [17, 1]
    return l1blk, l1bv, l2blk, l3w, l4w


def kernel(**inputs):
    import jax
    global LAST_DISPATCH_WALL

    x = np.asarray(inputs["x"], dtype=F32)
    edge_attr = np.asarray(inputs["edge_attr"], dtype=F32)
    ctx = _get_ctx(x, edge_attr, inputs["edge_index"])
    pp, disp, dev = ctx["pp"], ctx["disp"], ctx["dev"]

    params = []
    for li in (1, 2, 3, 4):
        params.append(
            _sample(
                inputs[f"l{li}_wmu"], inputs[f"l{li}_wrho"],
                inputs[f"l{li}_bmu"], inputs[f"l{li}_brho"],
                inputs[f"l{li}_eps_w"], inputs[f"l{li}_eps_b"],
            )
        )
    l1blk, l1bv, l2blk, l3w, l4w = _stationaries(params)
    wt = ctx["wtile"](l1blk, l1bv, l2blk, l3w, l4w)
    weights = dict(zip(("l1w", "l1b", "l2w", "l3w", "l4w"), wt))

    slow_path = pp["bad"].size > 0
    nodes_of = pp["nodes_of"]
    t0 = time.time()
    if not slow_path:
        args = dict(weights)
        args["ustatic"] = dev["ustatic"]
        args["nstat_s"] = dev["nstat_s"]
        args["sidx_pm"] = dev["sidx_pm"]
        args["uc"] = dev["uc0"]
        args["nstat_c"] = dev["nstat_c0"]
        for it in range(3):
            # last iteration uses the gather-free NEFF variant
            outs = (disp if it < 2 else ctx["disp_ng"])(args)
            if it < 2:
                # uc for the next iteration was gathered on device
                args["uc"] = outs["ucnext"]
                args["nstat_c"] = outs["cout"]
        cgrid = np.asarray(outs["cgrido"])     # fetch [32, NPB]
        LAST_DISPATCH_WALL = time.time() - t0
        c = np.empty(N, dtype=F32)
        c[nodes_of.reshape(-1)] = cgrid.reshape(-1)
        # pad positions point at node 0; restore its own value
        b0 = int(pp["node_bucket"][0]); p0 = int(pp["node_pos"][0])
        c[0] = cgrid[b0, p0]
    else:
        # correctness fallback: host roundtrip each iteration to fix bad nodes
        c = x[:, 2].astype(F32).copy()
        slot_src = pp["slot_src"]
        S2 = ctx["S2"]
        for _ in range(3):
            uc = np.empty((8, 4, S2), dtype=F32)
            for b in range(NB):
                cc, g = divmod(b, 4)
                uc[cc, g] = c[slot_src[b]]
            args = dict(weights)
            args["ustatic"] = dev["ustatic"]
            args["nstat_s"] = dev["nstat_s"]
            args["uc"] = uc.reshape(32, S2)
            args["nstat_c"] = c[nodes_of].astype(F32)
            cgrid = np.asarray(ctx["disp_ng"](args)["cout"])
            c_new = np.empty(N, dtype=F32)
            c_new[nodes_of.reshape(-1)] = cgrid.reshape(-1)
            b0 = int(pp["node_bucket"][0]); p0 = int(pp["node_pos"][0])
            c_new[0] = cgrid[b0, p0]
            c_new = _host_fix(c, c_new, pp, params, ctx["x2"], edge_attr)
            c = c_new
        LAST_DISPATCH_WALL = time.time() - t0

    out = np.empty((N, 3), dtype=F32)
    out[:, :2] = x[:, :2]
    out[:, 2] = c
    return out
